# revision 40
# baseline (speedup 1.0000x reference)
"""Trainium2 Bass kernel for nn_AttnMatching.

Reference computes:
    emb = emb_table[1:L+1]                      # [L, D]
    attn = einsum('ld,ntd->nlt', emb, self_attn)
    out  = einsum('nlt,t->nl', attn, value_w[0])

Reassociated (identical math, fp32):
    ctx[n, d] = sum_t value_w[t] * self_attn[n, t, d]    # [N, D]  (tiny)
    out[n, l] = sum_d ctx[n, d] * emb[l, d]              # [N, L]

Memory-bound: dominant traffic is streaming the 25.6 MB embedding table.
Sharding: vocab axis L split across 8 cores (6250 cols each),
self_attn/value_w replicated, no communication. Host-side marshalling
puts each tensor in its DMA-friendly layout:
  - emb shard pre-transposed to [D=128, Lsh] (contraction dim on
    partitions; large per-partition descriptors per chunk).
  - self_attn re-laid-out d-major as attn_dT[d, n*T+t] with value_w
    broadcast to [D, T] prepended -> one [128, 1700] region with
    contiguous 6.8 KB per-partition rows.

Per-core program (default raw bacc implementation, hand-rolled sems;
a TileContext variant is kept behind K_IMPL=tile):
  - attn+w bursts first on the sync HWDGE ring (ring FIFO gives it a
    solo full-rate window); sync then streams half the emb chunks,
    gpsimd (SWDGE) streams the rest once attnw has landed.
  - ctxT[d, n] built on the DVE: one fused multiply + free-dim-reduce
    (scalar_tensor_tensor accum_out) per batch row, pipelined behind
    the attnw sub-DMAs.
  - PE: dependency-free bf16 warmup matmuls hold the HAM at 2.4 GHz,
    then fp32 mains: lhsT=ctxT [D,16] stationary, rhs = emb chunks
    [D,<=512] -> PSUM [16,<=512] -> DVE copy -> chunked store DMA on
    the scalar ring.
  - Epilogue: sem-only all-engine barrier + semaphore range clear so
    the NEFF is safe to re-execute.
"""

import os

import numpy as np

L = 50000
D = 128
T = 100
N = 16
NCORES = 8
LSH = L // NCORES          # 6250 columns per core

# knobs (env-overridable for A/B profiling)
DMA_CHUNK = int(os.environ.get("K_DMA_CHUNK", "1024"))  # emb load granularity
MM_CHUNK = 512             # matmul moving-operand / PSUM bank limit
MM_DT = os.environ.get("K_MM_DT", "float32")  # matmul input dtype mode
NUM_DEVICES = int(os.environ.get("K_NUM_DEVICES", str(NCORES)))
N_WARMUP = int(os.environ.get("K_N_WARMUP", "8"))  # PE HAM warmup matmuls
IMPL = os.environ.get("K_IMPL", "raw")  # "tile" | "raw"

_cache = {}


def _chunks(total, step):
    return [(c0, min(c0 + step, total)) for c0 in range(0, total, step)]


def _build():
    import concourse.bacc as bacc
    import concourse.mybir as mybir
    import concourse.tile as tile

    mm_dt = getattr(mybir.dt, MM_DT)

    nc = bacc.Bacc(
        "TRN2",
        target_bir_lowering=False,
        debug=False,
        enable_asserts=True,
        num_devices=NUM_DEVICES,
    )

    embT = nc.dram_tensor("embT", [D, LSH], mm_dt, kind="ExternalInput").ap()
    attnw = nc.dram_tensor(
        "attnw", [T, N * D + 1], mybir.dt.float32, kind="ExternalInput"
    ).ap()
    out = nc.dram_tensor("out", [N, LSH], mybir.dt.float32, kind="ExternalOutput").ap()

    from concourse.tile_rust import add_dep_helper

    dma_chunks = _chunks(LSH, DMA_CHUNK)
    n_sync = (len(dma_chunks) + 1) // 2

    with tile.TileContext(nc) as tc:
        with (
            tc.tile_pool(name="consts", bufs=1) as consts,
            tc.tile_pool(name="embp", bufs=len(dma_chunks)) as embp,
            tc.tile_pool(name="outp", bufs=3) as outp,
            tc.tile_pool(name="psc", bufs=1, space="PSUM") as psc,
            tc.tile_pool(name="pso", bufs=4, space="PSUM") as pso,
        ):
            # attn+w upload, layout [w | n0..n15 blocks], split into 4
            # sub-DMAs issued FIRST on the sync ring: ring FIFO gives them
            # a solo full-rate burst before the emb stream, and the ctx
            # matmuls pipeline behind the sub-DMAs via subtile deps.
            attnw_tile = consts.tile([T, N * D + 1], mybir.dt.float32)
            attnw_bounds = [0, 513, 1025, 1537, 2049]
            attnw_last = None
            for a0, a1 in zip(attnw_bounds[:-1], attnw_bounds[1:]):
                attnw_last = nc.sync.dma_start(
                    attnw_tile[:, a0:a1], attnw[:, a0:a1]
                )

            # emb chunks: first half behind attnw on the sync ring (FIFO);
            # rest on the gpsimd ring, dep-delayed behind the attnw burst
            # so round-robin doesn't starve it.
            emb_tiles = []
            for ci, (c0, c1) in enumerate(dma_chunks):
                et = embp.tile(
                    [D, c1 - c0], mm_dt, tag="emb", name=f"emb_{c0}"
                )
                eng = nc.sync if ci < n_sync else nc.gpsimd
                dma = eng.dma_start(et[:, :], embT[:, c0:c1])
                if ci == n_sync:
                    add_dep_helper(
                        attnw_last.ins, dma.ins, sync=True,
                        reason="gpsimd emb stream waits for attnw burst",
                    )
                emb_tiles.append(et)

            # PE HAM warmup: dependency-free bf16 matmuls on a zeroed
            # scratch keep the PE at 2.4 GHz until real matmuls arrive.
            if N_WARMUP:
                wscr = consts.tile([D, D + MM_CHUNK], mybir.dt.bfloat16)
                nc.vector.memset(wscr[:, :], 0.0)
                ps_w = psc.tile(
                    [D, MM_CHUNK], mybir.dt.float32, tag="ps_warm", name="ps_warm"
                )
                for wi in range(N_WARMUP):
                    nc.tensor.matmul(
                        ps_w[:, :],
                        lhsT=wscr[:, :D],
                        rhs=wscr[:, D:],
                        start=True,
                        stop=True,
                    )

            # ctxT[d, n] = sum_t self_attn[n, t, d] * w[t]
            ps_ctx = psc.tile([D, N], mybir.dt.float32)
            for n in range(N):
                nc.tensor.matmul(
                    ps_ctx[:, n : n + 1],
                    lhsT=attnw_tile[:, 1 + n * D : 1 + (n + 1) * D],
                    rhs=attnw_tile[:, 0:1],
                    start=True,
                    stop=True,
                )
            ctxT = consts.tile([D, N], mm_dt)
            nc.vector.tensor_copy(ctxT[:, :], ps_ctx[:, :])
            ctxT_mm = ctxT[:, :]

            # out[n, c0:c1] = ctxT.T @ embT[:, c0:c1]
            for ci, (c0, c1) in enumerate(dma_chunks):
                ot = outp.tile([N, c1 - c0], mybir.dt.float32, tag="out", name=f"out_{c0}")
                for s0, s1 in _chunks(c1 - c0, MM_CHUNK):
                    ps = pso.tile(
                        [N, s1 - s0], mybir.dt.float32, tag="pso", name=f"ps_{c0}_{s0}"
                    )
                    nc.tensor.matmul(
                        ps[:, :],
                        lhsT=ctxT_mm,
                        rhs=emb_tiles[ci][:, s0:s1],
                        start=True,
                        stop=True,
                    )
                    nc.vector.tensor_copy(ot[:, s0:s1], ps[:, :])
                nc.scalar.dma_start(out[:, c0:c1], ot[:, :])

    nc.compile()
    return nc


def _build_raw():
    """Raw bacc (no TileContext): hand-rolled semaphores, same schedule as
    the Tile build but with a minimal prologue/epilogue."""
    import concourse.bacc as bacc
    import concourse.mybir as mybir

    f32 = mybir.dt.float32
    bf16 = mybir.dt.bfloat16

    nc = bacc.Bacc(
        "TRN2",
        target_bir_lowering=False,
        debug=False,
        enable_asserts=True,
        num_devices=NUM_DEVICES,
    )

    embT = nc.dram_tensor("embT", [D, LSH], f32, kind="ExternalInput").ap()
    # [D, T + N*T]: cols 0..T-1 = value_w broadcast over partitions,
    # cols T.. = self_attn in d-major layout attn_dT[d, n*T+t].
    AW = T + N * T
    attnw = nc.dram_tensor("attnw", [D, AW], f32, kind="ExternalInput").ap()
    out = nc.dram_tensor("out", [N, LSH], f32, kind="ExternalOutput").ap()

    dma_chunks = _chunks(LSH, DMA_CHUNK)
    n_chunks = len(dma_chunks)
    n_sync = (n_chunks + 1) // 2
    attnw_bounds = [0, T + 4 * T, T + 8 * T, T + 12 * T, AW]
    n_sub = len(attnw_bounds) - 1
    ctx_group_order = [0, 1, 2, 3]
    # gpsimd emb stream starts once this attnw sub-DMA has landed
    GP_DELAY_SUB = int(os.environ.get("K_GP_DELAY_SUB", str(n_sub - 1)))
    # global matmul list: (chunk_idx, abs_s0, abs_s1)
    mm_list = []
    for ci, (c0, c1) in enumerate(dma_chunks):
        for s0, s1 in _chunks(c1 - c0, MM_CHUNK):
            mm_list.append((ci, c0 + s0, c0 + s1))
    NPS = 4

    attnw_sb = nc.alloc_sbuf_tensor("attnw_sb", [D, AW], f32).ap()
    emb_sb = [
        nc.alloc_sbuf_tensor(f"emb_sb{ci}", [D, c1 - c0], f32).ap()
        for ci, (c0, c1) in enumerate(dma_chunks)
    ]
    out_sb = nc.alloc_sbuf_tensor("out_sb", [N, LSH], f32).ap()
    wscr = nc.alloc_sbuf_tensor("wscr", [D, D + MM_CHUNK], bf16).ap()
    ctxT = nc.alloc_sbuf_tensor("ctxT", [D, N], f32).ap()
    ctx_scr = nc.alloc_sbuf_tensor("ctx_scr", [D, N * T], f32).ap()
    ps_warm = nc.alloc_psum_tensor("ps_warm", [D, MM_CHUNK], f32).ap()
    ps_main = [
        nc.alloc_psum_tensor(f"ps_main{j}", [N, MM_CHUNK], f32).ap()
        for j in range(NPS)
    ]

    lda = [nc.alloc_semaphore(f"lda{g}") for g in range(n_sub)]
    lde = [nc.alloc_semaphore(f"lde{ci}") for ci in range(n_chunks)]
    z = nc.alloc_semaphore("z")
    cc = nc.alloc_semaphore("cc")
    mm = nc.alloc_semaphore("mm")
    cp = nc.alloc_semaphore("cp")
    st = nc.alloc_semaphore("st")
    all_sems = lda + lde + [z, cc, mm, cp, st]

    with nc.Block() as block:

        @block.sync
        def _(sync):
            for g, (a0, a1) in enumerate(zip(attnw_bounds[:-1], attnw_bounds[1:])):
                sync.dma_start(attnw_sb[:, a0:a1], attnw[:, a0:a1]).then_inc(
                    lda[g], 16
                )
            for ci in range(n_sync):
                c0, c1 = dma_chunks[ci]
                sync.dma_start(emb_sb[ci][:, :], embT[:, c0:c1]).then_inc(
                    lde[ci], 16
                )

        @block.gpsimd
        def _(gp):
            # don't compete with the attnw burst
            gp.wait_ge(lda[GP_DELAY_SUB], 16)
            for ci in range(n_sync, n_chunks):
                c0, c1 = dma_chunks[ci]
                gp.dma_start(emb_sb[ci][:, :], embT[:, c0:c1]).then_inc(
                    lde[ci], 16
                )

        @block.vector
        def _(v):
            # zero the PE warmup scratch first (PE gates on it)
            nc.vector.memset(wscr[:, :], 0.0).then_inc(z, 1)
            # ctxT[:, n] = sum_t attn_dT[:, n*T+t] * w[t] — one fused
            # multiply+freedim-reduce per n on the DVE.
            for gi in ctx_group_order:
              for nidx in range(gi * 4, gi * 4 + 4):
                if nidx % 4 == 0:
                    v.wait_ge(lda[nidx // 4], 16)
                inst = nc.vector.scalar_tensor_tensor(
                    ctx_scr[:, nidx * T : (nidx + 1) * T],
                    attnw_sb[:, T + nidx * T : T + (nidx + 1) * T],
                    1.0,
                    attnw_sb[:, 0:T],
                    op0=mybir.AluOpType.bypass,
                    op1=mybir.AluOpType.mult,
                    accum_out=ctxT[:, nidx : nidx + 1],
                )
            inst.then_inc(cc, 1)
            for s, (ci, s0, s1) in enumerate(mm_list):
                v.wait_ge(mm, s + 1)
                nc.vector.tensor_copy(
                    out_sb[:, s0:s1], ps_main[s % NPS][:, : s1 - s0]
                ).then_inc(cp, 1)

        @block.tensor
        def _(t):
            t.wait_ge(z, 1)
            for _wi in range(N_WARMUP):
                nc.tensor.matmul(
                    ps_warm[:, :],
                    lhsT=wscr[:, :D],
                    rhs=wscr[:, D:],
                    start=True,
                    stop=True,
                )
            t.wait_ge(cc, 1)
            prev_ci = -1
            for s, (ci, s0, s1) in enumerate(mm_list):
                if ci != prev_ci:
                    t.wait_ge(lde[ci], 16)
                    prev_ci = ci
                if s >= NPS:
                    t.wait_ge(cp, s - NPS + 1)
                c0 = dma_chunks[ci][0]
                nc.tensor.matmul(
                    ps_main[s % NPS][:, : s1 - s0],
                    lhsT=ctxT[:, :],
                    rhs=emb_sb[ci][:, s0 - c0 : s1 - c0],
                    start=True,
                    stop=True,
                ).then_inc(mm, 1)

        @block.scalar
        def _(sc):
            copies_done = 0
            for ci, (c0, c1) in enumerate(dma_chunks):
                copies_done += len(_chunks(c1 - c0, MM_CHUNK))
                sc.wait_ge(cp, copies_done)
                sc.dma_start(out[:, c0:c1], out_sb[:, c0:c1]).then_inc(st, 16)
            sc.wait_ge(st, 16 * n_chunks)

    # epilogue: quiesce engines, zero sems for re-execution safety
    nc.all_engine_barrier(sem_only=True)
    nc.clear_and_free_semaphores(all_sems)

    nc.compile()
    return nc


def _get_nc():
    if "nc" not in _cache:
        _cache["nc"] = _build_raw() if IMPL == "raw" else _build()
    return _cache["nc"]


def _make_in_maps(self_attn, emb_table, value_w):
    self_attn = np.asarray(self_attn, dtype=np.float32)
    value_w = np.asarray(value_w, dtype=np.float32)
    if IMPL == "raw":
        # [D, T + N*T]: value_w broadcast, then d-major self_attn
        attnw = np.empty((D, T + N * T), dtype=np.float32)
        attnw[:, :T] = value_w[0][None, :]
        attnw[:, T:] = self_attn.transpose(2, 0, 1).reshape(D, N * T)
    else:
        # [T, 1 + N*D]: value_w first, then transposed self_attn blocks
        attnw = np.empty((T, N * D + 1), dtype=np.float32)
        attnw[:, 0] = value_w[0]
        attnw[:, 1:] = self_attn.transpose(1, 0, 2).reshape(T, N * D)
    embT = np.asarray(emb_table, dtype=np.float32)[1 : L + 1].T  # [D, L]
    return [
        {
            "embT": np.ascontiguousarray(embT[:, k * LSH : (k + 1) * LSH]),
            "attnw": attnw,
        }
        for k in range(NCORES)
    ]


def run(self_attn, emb_table, value_w, trace=False):
    from concourse.bass_utils import run_bass_kernel_spmd

    nc = _get_nc()
    in_maps = _make_in_maps(self_attn, emb_table, value_w)
    res = run_bass_kernel_spmd(nc, in_maps, list(range(NCORES)), trace=trace)
    full = np.concatenate(
        [res.results[k]["out"] for k in range(NCORES)], axis=1
    ).astype(np.float32)
    return full, res


def kernel(self_attn, mat2, traj, emb_table, value_w):
    full, _ = run(self_attn, emb_table, value_w, trace=False)
    return full


# revision 41
# speedup vs baseline: 1.0396x; 1.0396x over previous
"""Trainium2 Bass kernel for nn_AttnMatching.

Reference computes:
    emb = emb_table[1:L+1]                      # [L, D]
    attn = einsum('ld,ntd->nlt', emb, self_attn)
    out  = einsum('nlt,t->nl', attn, value_w[0])

Reassociated (identical math, fp32):
    ctx[n, d] = sum_t value_w[t] * self_attn[n, t, d]    # [N, D]  (tiny)
    out[n, l] = sum_d ctx[n, d] * emb[l, d]              # [N, L]

Memory-bound: dominant traffic is streaming the 25.6 MB embedding table.
Sharding: vocab axis L split across 8 cores (6250 cols each),
self_attn/value_w replicated, no communication. Host-side marshalling
puts each tensor in its DMA-friendly layout:
  - emb shard pre-transposed to [D=128, Lsh] (contraction dim on
    partitions; large per-partition descriptors per chunk).
  - self_attn re-laid-out d-major as attn_dT[d, n*T+t] with value_w
    broadcast to [D, T] prepended -> one [128, 1700] region with
    contiguous 6.8 KB per-partition rows.

Per-core program (default raw bacc implementation, hand-rolled sems;
a TileContext variant is kept behind K_IMPL=tile):
  - attn+w bursts first on the sync HWDGE ring (ring FIFO gives it a
    solo full-rate window); sync then streams half the emb chunks,
    gpsimd (SWDGE) streams the rest once attnw has landed.
  - ctxT[d, n] built on the DVE: one fused multiply + free-dim-reduce
    (scalar_tensor_tensor accum_out) per batch row, pipelined behind
    the attnw sub-DMAs.
  - PE: dependency-free bf16 warmup matmuls hold the HAM at 2.4 GHz,
    then fp32 mains: lhsT=ctxT [D,16] stationary, rhs = emb chunks
    [D,<=512] -> PSUM [16,<=512] -> DVE copy -> chunked store DMA on
    the scalar ring.
  - Epilogue: sem-only all-engine barrier + semaphore range clear so
    the NEFF is safe to re-execute.
"""

import os

import numpy as np

L = 50000
D = 128
T = 100
N = 16
NCORES = 8
LSH = L // NCORES          # 6250 columns per core

# knobs (env-overridable for A/B profiling)
DMA_CHUNK = int(os.environ.get("K_DMA_CHUNK", "1024"))  # emb load granularity
MM_CHUNK = 512             # matmul moving-operand / PSUM bank limit
MM_DT = os.environ.get("K_MM_DT", "float32")  # matmul input dtype mode
NUM_DEVICES = int(os.environ.get("K_NUM_DEVICES", str(NCORES)))
N_WARMUP = int(os.environ.get("K_N_WARMUP", "8"))  # PE HAM warmup matmuls
IMPL = os.environ.get("K_IMPL", "raw")  # "tile" | "raw"

_cache = {}


def _chunks(total, step):
    return [(c0, min(c0 + step, total)) for c0 in range(0, total, step)]


def _build():
    import concourse.bacc as bacc
    import concourse.mybir as mybir
    import concourse.tile as tile

    mm_dt = getattr(mybir.dt, MM_DT)

    nc = bacc.Bacc(
        "TRN2",
        target_bir_lowering=False,
        debug=False,
        enable_asserts=True,
        num_devices=NUM_DEVICES,
    )

    embT = nc.dram_tensor("embT", [D, LSH], mm_dt, kind="ExternalInput").ap()
    attnw = nc.dram_tensor(
        "attnw", [T, N * D + 1], mybir.dt.float32, kind="ExternalInput"
    ).ap()
    out = nc.dram_tensor("out", [N, LSH], mybir.dt.float32, kind="ExternalOutput").ap()

    from concourse.tile_rust import add_dep_helper

    dma_chunks = _chunks(LSH, DMA_CHUNK)
    n_sync = (len(dma_chunks) + 1) // 2

    with tile.TileContext(nc) as tc:
        with (
            tc.tile_pool(name="consts", bufs=1) as consts,
            tc.tile_pool(name="embp", bufs=len(dma_chunks)) as embp,
            tc.tile_pool(name="outp", bufs=3) as outp,
            tc.tile_pool(name="psc", bufs=1, space="PSUM") as psc,
            tc.tile_pool(name="pso", bufs=4, space="PSUM") as pso,
        ):
            # attn+w upload, layout [w | n0..n15 blocks], split into 4
            # sub-DMAs issued FIRST on the sync ring: ring FIFO gives them
            # a solo full-rate burst before the emb stream, and the ctx
            # matmuls pipeline behind the sub-DMAs via subtile deps.
            attnw_tile = consts.tile([T, N * D + 1], mybir.dt.float32)
            attnw_bounds = [0, 513, 1025, 1537, 2049]
            attnw_last = None
            for a0, a1 in zip(attnw_bounds[:-1], attnw_bounds[1:]):
                attnw_last = nc.sync.dma_start(
                    attnw_tile[:, a0:a1], attnw[:, a0:a1]
                )

            # emb chunks: first half behind attnw on the sync ring (FIFO);
            # rest on the gpsimd ring, dep-delayed behind the attnw burst
            # so round-robin doesn't starve it.
            emb_tiles = []
            for ci, (c0, c1) in enumerate(dma_chunks):
                et = embp.tile(
                    [D, c1 - c0], mm_dt, tag="emb", name=f"emb_{c0}"
                )
                eng = nc.sync if ci < n_sync else nc.gpsimd
                dma = eng.dma_start(et[:, :], embT[:, c0:c1])
                if ci == n_sync:
                    add_dep_helper(
                        attnw_last.ins, dma.ins, sync=True,
                        reason="gpsimd emb stream waits for attnw burst",
                    )
                emb_tiles.append(et)

            # PE HAM warmup: dependency-free bf16 matmuls on a zeroed
            # scratch keep the PE at 2.4 GHz until real matmuls arrive.
            if N_WARMUP:
                wscr = consts.tile([D, D + MM_CHUNK], mybir.dt.bfloat16)
                nc.vector.memset(wscr[:, :], 0.0)
                ps_w = psc.tile(
                    [D, MM_CHUNK], mybir.dt.float32, tag="ps_warm", name="ps_warm"
                )
                for wi in range(N_WARMUP):
                    nc.tensor.matmul(
                        ps_w[:, :],
                        lhsT=wscr[:, :D],
                        rhs=wscr[:, D:],
                        start=True,
                        stop=True,
                    )

            # ctxT[d, n] = sum_t self_attn[n, t, d] * w[t]
            ps_ctx = psc.tile([D, N], mybir.dt.float32)
            for n in range(N):
                nc.tensor.matmul(
                    ps_ctx[:, n : n + 1],
                    lhsT=attnw_tile[:, 1 + n * D : 1 + (n + 1) * D],
                    rhs=attnw_tile[:, 0:1],
                    start=True,
                    stop=True,
                )
            ctxT = consts.tile([D, N], mm_dt)
            nc.vector.tensor_copy(ctxT[:, :], ps_ctx[:, :])
            ctxT_mm = ctxT[:, :]

            # out[n, c0:c1] = ctxT.T @ embT[:, c0:c1]
            for ci, (c0, c1) in enumerate(dma_chunks):
                ot = outp.tile([N, c1 - c0], mybir.dt.float32, tag="out", name=f"out_{c0}")
                for s0, s1 in _chunks(c1 - c0, MM_CHUNK):
                    ps = pso.tile(
                        [N, s1 - s0], mybir.dt.float32, tag="pso", name=f"ps_{c0}_{s0}"
                    )
                    nc.tensor.matmul(
                        ps[:, :],
                        lhsT=ctxT_mm,
                        rhs=emb_tiles[ci][:, s0:s1],
                        start=True,
                        stop=True,
                    )
                    nc.vector.tensor_copy(ot[:, s0:s1], ps[:, :])
                nc.scalar.dma_start(out[:, c0:c1], ot[:, :])

    nc.compile()
    return nc


def _build_raw():
    """Raw bacc (no TileContext): hand-rolled semaphores, same schedule as
    the Tile build but with a minimal prologue/epilogue."""
    import concourse.bacc as bacc
    import concourse.mybir as mybir

    f32 = mybir.dt.float32
    bf16 = mybir.dt.bfloat16

    nc = bacc.Bacc(
        "TRN2",
        target_bir_lowering=False,
        debug=False,
        enable_asserts=True,
        num_devices=NUM_DEVICES,
    )

    embT = nc.dram_tensor("embT", [D, LSH], f32, kind="ExternalInput").ap()
    # [D, T + N*T]: cols 0..T-1 = value_w broadcast over partitions,
    # cols T.. = self_attn in d-major layout attn_dT[d, n*T+t].
    AW = T + N * T
    attnw = nc.dram_tensor("attnw", [D, AW], f32, kind="ExternalInput").ap()
    out = nc.dram_tensor("out", [N, LSH], f32, kind="ExternalOutput").ap()

    # first emb chunk small (one matmul's worth): it queues behind the
    # attnw burst on the sync ring and gates the first main matmul, so
    # keep its transfer short.
    dma_chunks = [(0, MM_CHUNK)] + [
        (c0 + MM_CHUNK, c1 + MM_CHUNK) for c0, c1 in _chunks(LSH - MM_CHUNK, DMA_CHUNK)
    ]
    n_chunks = len(dma_chunks)
    n_sync = (n_chunks + 1) // 2
    attnw_bounds = [0, T + 4 * T, T + 8 * T, T + 12 * T, AW]
    n_sub = len(attnw_bounds) - 1
    ctx_group_order = [0, 1, 2, 3]
    # gpsimd emb stream starts once this attnw sub-DMA has landed
    GP_DELAY_SUB = int(os.environ.get("K_GP_DELAY_SUB", str(n_sub - 1)))
    # global matmul list: (chunk_idx, abs_s0, abs_s1)
    mm_list = []
    for ci, (c0, c1) in enumerate(dma_chunks):
        for s0, s1 in _chunks(c1 - c0, MM_CHUNK):
            mm_list.append((ci, c0 + s0, c0 + s1))
    NPS = 4

    attnw_sb = nc.alloc_sbuf_tensor("attnw_sb", [D, AW], f32).ap()
    emb_sb = [
        nc.alloc_sbuf_tensor(f"emb_sb{ci}", [D, c1 - c0], f32).ap()
        for ci, (c0, c1) in enumerate(dma_chunks)
    ]
    out_sb = nc.alloc_sbuf_tensor("out_sb", [N, LSH], f32).ap()
    wscr = nc.alloc_sbuf_tensor("wscr", [D, D + MM_CHUNK], bf16).ap()
    ctxT = nc.alloc_sbuf_tensor("ctxT", [D, N], f32).ap()
    ctx_scr = nc.alloc_sbuf_tensor("ctx_scr", [D, N * T], f32).ap()
    ps_warm = nc.alloc_psum_tensor("ps_warm", [D, MM_CHUNK], f32).ap()
    ps_main = [
        nc.alloc_psum_tensor(f"ps_main{j}", [N, MM_CHUNK], f32).ap()
        for j in range(NPS)
    ]

    lda = [nc.alloc_semaphore(f"lda{g}") for g in range(n_sub)]
    lde = [nc.alloc_semaphore(f"lde{ci}") for ci in range(n_chunks)]
    z = nc.alloc_semaphore("z")
    cc = nc.alloc_semaphore("cc")
    mm = nc.alloc_semaphore("mm")
    cp = nc.alloc_semaphore("cp")
    st = nc.alloc_semaphore("st")
    all_sems = lda + lde + [z, cc, mm, cp, st]

    with nc.Block() as block:

        @block.sync
        def _(sync):
            for g, (a0, a1) in enumerate(zip(attnw_bounds[:-1], attnw_bounds[1:])):
                sync.dma_start(attnw_sb[:, a0:a1], attnw[:, a0:a1]).then_inc(
                    lda[g], 16
                )
            for ci in range(n_sync):
                c0, c1 = dma_chunks[ci]
                sync.dma_start(emb_sb[ci][:, :], embT[:, c0:c1]).then_inc(
                    lde[ci], 16
                )

        @block.gpsimd
        def _(gp):
            # don't compete with the attnw burst
            gp.wait_ge(lda[GP_DELAY_SUB], 16)
            for ci in range(n_sync, n_chunks):
                c0, c1 = dma_chunks[ci]
                gp.dma_start(emb_sb[ci][:, :], embT[:, c0:c1]).then_inc(
                    lde[ci], 16
                )

        @block.vector
        def _(v):
            # zero the PE warmup scratch first (PE gates on it)
            nc.vector.memset(wscr[:, :], 0.0).then_inc(z, 1)
            # ctxT[:, n] = sum_t attn_dT[:, n*T+t] * w[t] — one fused
            # multiply+freedim-reduce per n on the DVE.
            for gi in ctx_group_order:
              for nidx in range(gi * 4, gi * 4 + 4):
                if nidx % 4 == 0:
                    v.wait_ge(lda[nidx // 4], 16)
                inst = nc.vector.scalar_tensor_tensor(
                    ctx_scr[:, nidx * T : (nidx + 1) * T],
                    attnw_sb[:, T + nidx * T : T + (nidx + 1) * T],
                    1.0,
                    attnw_sb[:, 0:T],
                    op0=mybir.AluOpType.bypass,
                    op1=mybir.AluOpType.mult,
                    accum_out=ctxT[:, nidx : nidx + 1],
                )
            inst.then_inc(cc, 1)
            for s, (ci, s0, s1) in enumerate(mm_list):
                v.wait_ge(mm, s + 1)
                nc.vector.tensor_copy(
                    out_sb[:, s0:s1], ps_main[s % NPS][:, : s1 - s0]
                ).then_inc(cp, 1)

        @block.tensor
        def _(t):
            t.wait_ge(z, 1)
            for _wi in range(N_WARMUP):
                nc.tensor.matmul(
                    ps_warm[:, :],
                    lhsT=wscr[:, :D],
                    rhs=wscr[:, D:],
                    start=True,
                    stop=True,
                )
            t.wait_ge(cc, 1)
            prev_ci = -1
            for s, (ci, s0, s1) in enumerate(mm_list):
                if ci != prev_ci:
                    t.wait_ge(lde[ci], 16)
                    prev_ci = ci
                if s >= NPS:
                    t.wait_ge(cp, s - NPS + 1)
                c0 = dma_chunks[ci][0]
                nc.tensor.matmul(
                    ps_main[s % NPS][:, : s1 - s0],
                    lhsT=ctxT[:, :],
                    rhs=emb_sb[ci][:, s0 - c0 : s1 - c0],
                    start=True,
                    stop=True,
                ).then_inc(mm, 1)

        @block.scalar
        def _(sc):
            copies_done = 0
            for ci, (c0, c1) in enumerate(dma_chunks):
                copies_done += len(_chunks(c1 - c0, MM_CHUNK))
                sc.wait_ge(cp, copies_done)
                sc.dma_start(out[:, c0:c1], out_sb[:, c0:c1]).then_inc(st, 16)
            sc.wait_ge(st, 16 * n_chunks)

    # epilogue: quiesce engines, zero sems for re-execution safety
    nc.all_engine_barrier(sem_only=True)
    nc.clear_and_free_semaphores(all_sems)

    nc.compile()
    return nc


def _get_nc():
    if "nc" not in _cache:
        _cache["nc"] = _build_raw() if IMPL == "raw" else _build()
    return _cache["nc"]


def _make_in_maps(self_attn, emb_table, value_w):
    self_attn = np.asarray(self_attn, dtype=np.float32)
    value_w = np.asarray(value_w, dtype=np.float32)
    if IMPL == "raw":
        # [D, T + N*T]: value_w broadcast, then d-major self_attn
        attnw = np.empty((D, T + N * T), dtype=np.float32)
        attnw[:, :T] = value_w[0][None, :]
        attnw[:, T:] = self_attn.transpose(2, 0, 1).reshape(D, N * T)
    else:
        # [T, 1 + N*D]: value_w first, then transposed self_attn blocks
        attnw = np.empty((T, N * D + 1), dtype=np.float32)
        attnw[:, 0] = value_w[0]
        attnw[:, 1:] = self_attn.transpose(1, 0, 2).reshape(T, N * D)
    embT = np.asarray(emb_table, dtype=np.float32)[1 : L + 1].T  # [D, L]
    return [
        {
            "embT": np.ascontiguousarray(embT[:, k * LSH : (k + 1) * LSH]),
            "attnw": attnw,
        }
        for k in range(NCORES)
    ]


def run(self_attn, emb_table, value_w, trace=False):
    from concourse.bass_utils import run_bass_kernel_spmd

    nc = _get_nc()
    in_maps = _make_in_maps(self_attn, emb_table, value_w)
    res = run_bass_kernel_spmd(nc, in_maps, list(range(NCORES)), trace=trace)
    full = np.concatenate(
        [res.results[k]["out"] for k in range(NCORES)], axis=1
    ).astype(np.float32)
    return full, res


def kernel(self_attn, mat2, traj, emb_table, value_w):
    full, _ = run(self_attn, emb_table, value_w, trace=False)
    return full
